# revision 9
# baseline (speedup 1.0000x reference)
"""Trainium2 Bass kernel for nn_Block_48223892799907 (attention + MoE block).

Sharding (8 cores, SPMD-uniform program):
  - Tokens: core c owns global token rows [c*512, (c+1)*512).
  - Attention: head-parallel via AllToAll (core c computes heads {2c,2c+1}
    over the full causal span for both batches).
  - MoE: expert-parallel (core c owns experts {2c,2c+1}); on-device dispatch
    (per-partition scan + prefix matmul + slot-inversion matmul + indirect
    DMA gather / CCE-add scatter), capacity 640/expert.
  - Shared expert: sharded over the hidden dim (256 cols per core).
  - Host: shards/casts inputs, sums per-core partials (unshard).
"""
import numpy as np
import ml_dtypes

import concourse.bass as bass
import concourse.bacc as bacc
import concourse.mybir as mybir
import concourse.tile as tile
from concourse.bass_utils import run_bass_kernel_spmd
from concourse.masks import make_identity, make_upper_triangular

F32 = mybir.dt.float32
BF16 = mybir.dt.bfloat16
I32 = mybir.dt.int32
AF = mybir.ActivationFunctionType
OP = mybir.AluOpType
AX = mybir.AxisListType

B, T, C, H, DH, E, TOPK = 2, 2048, 1024, 16, 64, 16, 2
NCORES = 8
OWN = B * T // NCORES          # 512
NTOK = B * T                   # 4096
CAP = 640
CAPT = CAP // 128              # 5
EPC = 2
HIDS = 2 * C // NCORES         # 256
EPS = 1e-5
ALPHA = 0.01
P = 128


def build_nc():
    nc = bacc.Bacc("TRN2", target_bir_lowering=False, num_devices=NCORES)

    # ------------- external inputs (per-core staged by host) -------------
    x_own = nc.dram_tensor("x_own", [OWN, C], F32, kind="ExternalInput")
    ln1_w = nc.dram_tensor("ln1_w", [1, C], F32, kind="ExternalInput")
    ln2_wt = nc.dram_tensor("ln2_wt", [P, 8], F32, kind="ExternalInput")
    w_attn_b = nc.dram_tensor("w_attn_b", [C, 3 * C], BF16, kind="ExternalInput")
    b_attn = nc.dram_tensor("b_attn", [1, 3 * C], BF16, kind="ExternalInput")
    w_proj_b = nc.dram_tensor("w_proj_b", [C, C], BF16, kind="ExternalInput")
    b_proj = nc.dram_tensor("b_proj", [1, C], BF16, kind="ExternalInput")
    gate_wt = nc.dram_tensor("gate_wt", [C, E], F32, kind="ExternalInput")
    gate_bias = nc.dram_tensor("gate_bias", [1, E], F32, kind="ExternalInput")
    cos_own = nc.dram_tensor("cos_own", [OWN, DH // 2], F32, kind="ExternalInput")
    sin_own = nc.dram_tensor("sin_own", [OWN, DH // 2], F32, kind="ExternalInput")
    exp_wf_b = nc.dram_tensor("exp_wf_b", [EPC, C, 2 * C], BF16, kind="ExternalInput")
    exp_wp_b = nc.dram_tensor("exp_wp_b", [EPC, 2 * C, C], BF16, kind="ExternalInput")
    exp_bf_t = nc.dram_tensor("exp_bf_t", [P, EPC * 16], F32, kind="ExternalInput")
    exp_bp = nc.dram_tensor("exp_bp", [EPC, C], BF16, kind="ExternalInput")
    sh_wf_b = nc.dram_tensor("sh_wf_b", [C, HIDS], BF16, kind="ExternalInput")
    sh_wp_b = nc.dram_tensor("sh_wp_b", [HIDS, C], BF16, kind="ExternalInput")
    sh_bf_t = nc.dram_tensor("sh_bf_t", [P, 2], F32, kind="ExternalInput")
    sh_bp8 = nc.dram_tensor("sh_bp8", [1, C], BF16, kind="ExternalInput")

    # ------------- outputs -------------
    out_partial = nc.dram_tensor("out_partial", [NTOK, C], F32, kind="ExternalOutput")
    x2_out = nc.dram_tensor("x2_out", [OWN, C], F32, kind="ExternalOutput")
    lb_out = nc.dram_tensor("lb_out", [1, 1], F32, kind="ExternalOutput")

    # ------------- internal DRAM -------------
    rg = [list(range(NCORES))]

    def cc_pair(name, shape_in, shape_out, dtype, shared=False):
        i = nc.dram_tensor(name + "_i", shape_in, dtype)
        o = nc.dram_tensor(name + "_o", shape_out, dtype,
                           addr_space="Shared" if shared else "Local")
        return i, o

    a2aq_i, a2aq_o = cc_pair("a2aq", [C, OWN], [C, OWN], BF16)
    a2ak_i, a2ak_o = cc_pair("a2ak", [C, OWN], [C, OWN], BF16)
    a2av_i, a2av_o = cc_pair("a2av", [NTOK, 130], [NTOK, 130], BF16)
    a2ay_i, a2ay_o = cc_pair("a2ay", [C, OWN], [C, OWN], BF16)
    a2aw_i, a2aw_o = cc_pair("a2aw", [NTOK, EPC], [NTOK, EPC], F32)
    agst_i, agst_o = cc_pair("agst", [OWN, 2 * E], [NTOK, 2 * E], F32, shared=True)
    agh2_i, agh2_o = cc_pair("agh2", [OWN, C], [NTOK, C], BF16, shared=True)
    agh2t_i, agh2t_o = cc_pair("agh2t", [C, OWN], [NCORES * C, OWN], BF16, shared=True)

    ids_dram = nc.dram_tensor("ids_dram", [EPC * CAP], F32)
    wsl_dram = nc.dram_tensor("wsl_dram", [EPC * CAP], F32)
    rcp_dram = nc.dram_tensor("rcp_dram", [512], F32)
    sc2_dram = nc.dram_tensor("sc2_dram", [P, 4], F32)

    with tile.TileContext(nc) as tc:
        with (
            tc.tile_pool(name="consts", bufs=1) as cpool,
        ):
            # ---------------- constants ----------------
            ident = cpool.tile([P, P], BF16)
            make_identity(nc, ident[:])
            identf = cpool.tile([P, P], F32)
            make_identity(nc, identf[:])
            ustrict = cpool.tile([P, P], F32)
            make_upper_triangular(nc, ustrict[:], val=1.0, diag=False)
            ones_row = cpool.tile([1, P], BF16)
            nc.vector.memset(ones_row[:], 1.0)
            onesb = cpool.tile([P, 1], F32)
            nc.vector.memset(onesb[:], 1.0)
            eps_col = cpool.tile([P, 1], F32)
            nc.vector.memset(eps_col[:], EPS)
            diagmask = []
            for j in range(4):
                m = cpool.tile([P, 512], BF16, tag=f"dm{j}")
                nc.gpsimd.memset(m[:], 0.0)
                # keep when (kk + 128j) <= qq  i.e. fill 1.0 where iota<0 is False:
                # iota = -qq + kk + 128j ; fill where iota <= 0
                nc.gpsimd.affine_select(
                    out=m[:], in_=m[:], compare_op=OP.is_gt, fill=1.0,
                    base=128 * j, pattern=[[-1, 512]], channel_multiplier=1)
                diagmask.append(m)
            iota_cap_i = cpool.tile([P, CAP], I32)
            nc.gpsimd.iota(iota_cap_i[:], pattern=[[1, CAP]], base=0, channel_multiplier=0)
            iota_cap = cpool.tile([P, CAP], F32)
            nc.vector.tensor_copy(iota_cap[:], iota_cap_i[:])
            iota_tok_i = cpool.tile([P, 32], I32)
            nc.gpsimd.iota(iota_tok_i[:], pattern=[[1, 32]], base=0, channel_multiplier=32)
            iota_tok = cpool.tile([P, 32], F32)
            nc.vector.tensor_copy(iota_tok[:], iota_tok_i[:])
            ln1_b = cpool.tile([P, C], F32)
            nc.sync.dma_start(out=ln1_b[:], in_=ln1_w[0:1, :].to_broadcast((P, C)))
            gbias_b = cpool.tile([P, E], F32)
            nc.sync.dma_start(out=gbias_b[:], in_=gate_bias[0:1, :].to_broadcast((P, E)))

            # ============ attention phases ============
            with (
                tc.tile_pool(name="pAF", bufs=1) as pa,
                tc.tile_pool(name="pLoop", bufs=2) as pl,
                tc.tile_pool(name="psAF", bufs=2, space="PSUM") as psa,
            ):
                # --- rmsnorm1 -> h bf16 -> h_T ---
                x_sb = pa.tile([P, 4, C], F32)
                nc.sync.dma_start(out=x_sb[:],
                                  in_=x_own[:, :].rearrange("(t p) c -> p t c", p=P))
                h_bf = pa.tile([P, 4, C], BF16)
                for tt in range(4):
                    sq = pl.tile([P, C], BF16, tag="sq")
                    ssq = pl.tile([P, 1], F32, tag="ssq")
                    nc.scalar.activation(sq[:], x_sb[:, tt, :], AF.Square,
                                         scale=float(1.0 / 32.0), accum_out=ssq[:])
                    lnv = pl.tile([P, 1], F32, tag="lnv")
                    nc.scalar.activation(lnv[:], ssq[:], AF.Ln, bias=eps_col[:, :1])
                    sc = pl.tile([P, 1], F32, tag="sc")
                    nc.scalar.activation(sc[:], lnv[:], AF.Exp, scale=-0.5)
                    tmp = pl.tile([P, C], F32, tag="tmpn")
                    nc.vector.tensor_scalar(out=tmp[:], in0=x_sb[:, tt, :],
                                            scalar1=sc[:, :1], scalar2=None, op0=OP.mult)
                    nc.vector.tensor_mul(h_bf[:, tt, :], tmp[:], ln1_b[:])
                h_T = pa.tile([P, 8, OWN], BF16)
                for tt in range(4):
                    for kb in range(8):
                        tp = psa.tile([P, P], BF16, space="PSUM", tag="tp")
                        nc.tensor.transpose(tp[:], h_bf[:, tt, kb * P:(kb + 1) * P], ident[:])
                        nc.scalar.copy(h_T[:, kb, tt * P:(tt + 1) * P], tp[:])

                # --- QKV (token-major out) ---
                ba_sb = pa.tile([1, 3 * C], BF16)
                nc.sync.dma_start(out=ba_sb[:], in_=b_attn[:, :])
                q_tok = pa.tile([P, 4, C], BF16)
                k_tok = pa.tile([P, 4, C], BF16)
                v_sb = pa.tile([P, 4, H * 65], BF16)
                nc.vector.memset(v_sb[:], 1.0)
                for cg in range(6):
                    wa = pl.tile([P, 8, 512], BF16, tag="wa")
                    nc.sync.dma_start(
                        out=wa[:],
                        in_=w_attn_b[:, cg * 512:(cg + 1) * 512]
                        .rearrange("(k p) n -> p k n", p=P))
                    for tt in range(4):
                        ps = psa.tile([P, 512], F32, space="PSUM", tag="qkv")
                        for k in range(8):
                            nc.tensor.matmul(ps[:], lhsT=h_T[:, k, tt * P:(tt + 1) * P],
                                             rhs=wa[:, k, :],
                                             start=(k == 0), stop=False)
                        nc.tensor.matmul(ps[:], lhsT=ones_row[:1, :],
                                         rhs=ba_sb[:1, cg * 512:(cg + 1) * 512],
                                         start=False, stop=True)
                        if cg < 2:
                            nc.scalar.copy(q_tok[:, tt, cg * 512:(cg + 1) * 512], ps[:])
                        elif cg < 4:
                            nc.scalar.copy(k_tok[:, tt, (cg - 2) * 512:(cg - 1) * 512], ps[:])
                        else:
                            hb = (cg - 4) * 8
                            nc.scalar.copy(
                                v_sb[:, tt, :].rearrange("p (h d) -> p h d", d=65)
                                [:, hb:hb + 8, 0:64],
                                ps[:].rearrange("p (h d) -> p h d", d=64))

                # --- RoPE (token-major, strided views) ---
                cs_f = pa.tile([P, 4, DH // 2], F32, tag="cosf")
                nc.sync.dma_start(out=cs_f[:],
                                  in_=cos_own[:, :].rearrange("(t p) c -> p t c", p=P))
                sn_f = pa.tile([P, 4, DH // 2], F32, tag="sinf")
                nc.sync.dma_start(out=sn_f[:],
                                  in_=sin_own[:, :].rearrange("(t p) c -> p t c", p=P))
                cs = pa.tile([P, 4, DH // 2], BF16, tag="cosb")
                nc.vector.tensor_copy(cs[:], cs_f[:])
                sn = pa.tile([P, 4, DH // 2], BF16, tag="sinb")
                nc.vector.tensor_copy(sn[:], sn_f[:])
                for tt in range(4):
                    cosb = cs[:, tt:tt + 1, :].to_broadcast([P, H, DH // 2])
                    sinb = sn[:, tt:tt + 1, :].to_broadcast([P, H, DH // 2])
                    for src, eng in ((q_tok, nc.vector), (k_tok, nc.gpsimd)):
                        sv = src[:, tt, :].rearrange("p (h c two) -> p h c two",
                                                     two=2, c=DH // 2)
                        t1 = pl.tile([P, H, DH // 2], BF16, tag="rope1")
                        t2 = pl.tile([P, H, DH // 2], BF16, tag="rope2")
                        t3 = pl.tile([P, H, DH // 2], BF16, tag="rope3")
                        t4 = pl.tile([P, H, DH // 2], BF16, tag="rope4")
                        eng.tensor_tensor(out=t1[:], in0=sv[:, :, :, 0], in1=cosb, op=OP.mult)
                        eng.tensor_tensor(out=t2[:], in0=sv[:, :, :, 1], in1=sinb, op=OP.mult)
                        eng.tensor_tensor(out=t3[:], in0=sv[:, :, :, 0], in1=sinb, op=OP.mult)
                        eng.tensor_tensor(out=t4[:], in0=sv[:, :, :, 1], in1=cosb, op=OP.mult)
                        eng.tensor_tensor(out=sv[:, :, :, 0], in0=t1[:], in1=t2[:],
                                          op=OP.subtract)
                        eng.tensor_tensor(out=sv[:, :, :, 1], in0=t3[:], in1=t4[:], op=OP.add)
                qT = pa.tile([P, 8, OWN], BF16, tag="h_T")
                kT = pa.tile([P, 8, OWN], BF16, tag="h_bf")
                for src, dstT in ((q_tok, qT), (k_tok, kT)):
                    for tt in range(4):
                        for kb in range(8):
                            tp = psa.tile([P, P], BF16, space="PSUM", tag="tp")
                            nc.tensor.transpose(tp[:], src[:, tt, kb * P:(kb + 1) * P],
                                                ident[:])
                            nc.scalar.copy(dstT[:, kb, tt * P:(tt + 1) * P], tp[:])

                # --- A2A q, k, v ---
                nc.sync.dma_start(out=a2aq_i[:, :].rearrange("(r p) t -> p r t", p=P),
                                  in_=qT[:])
                nc.sync.dma_start(out=a2ak_i[:, :].rearrange("(r p) t -> p r t", p=P),
                                  in_=kT[:])
                for r in range(NCORES):
                    nc.sync.dma_start(
                        out=a2av_i[r * OWN:(r + 1) * OWN, :]
                        .rearrange("(t p) d -> p t d", p=P),
                        in_=v_sb[:, :, r * 130:(r + 1) * 130])
                for ci, co in ((a2aq_i, a2aq_o), (a2ak_i, a2ak_o), (a2av_i, a2av_o)):
                    nc.gpsimd.collective_compute("AllToAll", OP.bypass, replica_groups=rg,
                                                 ins=[ci[:, :]], outs=[co[:, :]])
                qm = pa.tile([P, NTOK], BF16, tag="q_tok")
                km = pa.tile([P, NTOK], BF16, tag="k_tok")
                nc.sync.dma_start(out=qm[:].rearrange("p (r t) -> p r t", r=NCORES),
                                  in_=a2aq_o[:, :].rearrange("(r p) t -> p r t", p=P))
                nc.sync.dma_start(out=km[:].rearrange("p (r t) -> p r t", r=NCORES),
                                  in_=a2ak_o[:, :].rearrange("(r p) t -> p r t", p=P))
                vm = pa.tile([P, 32, 130], BF16, tag="v_sb")
                nc.sync.dma_start(out=vm[:],
                                  in_=a2av_o[:, :].rearrange("(tt p) d -> p tt d", p=P))

                # --- causal attention: my 2 heads x 2 batches ---
                yT0 = pa.tile([64, NTOK], BF16, tag="yT0")
                yT1 = pa.tile([64, NTOK], BF16, tag="yT1")
                ytiles = (yT0, yT1)
                for bb in range(2):
                    for hj in range(2):
                        hrow = hj * 64
                        vcol = hj * 65
                        for qc in range(4):
                            qoff = bb * 2048 + qc * 512
                            py = psa.tile([65, 512], F32, space="PSUM", tag="py")
                            nkt = 4 * (qc + 1)
                            for kt in range(nkt):
                                koff = bb * 2048 + kt * P
                                pss = psa.tile([P, 512], F32, space="PSUM", tag="pss")
                                nc.tensor.matmul(
                                    pss[:], lhsT=km[hrow:hrow + 64, koff:koff + P],
                                    rhs=qm[hrow:hrow + 64, qoff:qoff + 512],
                                    start=True, stop=True)
                                et = pl.tile([P, 512], BF16, tag="exp")
                                nc.scalar.activation(et[:], pss[:], AF.Exp, scale=0.125)
                                if kt >= 4 * qc:
                                    nc.vector.tensor_mul(et[:], et[:],
                                                         diagmask[kt - 4 * qc][:])
                                nc.tensor.matmul(
                                    py[:], lhsT=vm[:, bb * 16 + kt, vcol:vcol + 65],
                                    rhs=et[:], start=(kt == 0), stop=(kt == nkt - 1))
                            rcp = pl.tile([1, 512], F32, tag="rcp")
                            nc.vector.reciprocal(rcp[:], py[64:65, :])
                            nc.sync.dma_start(out=rcp_dram[None, :], in_=rcp[:])
                            rcpb = pl.tile([64, 512], F32, tag="rcpb")
                            nc.sync.dma_start(out=rcpb[:],
                                              in_=rcp_dram[None, :].to_broadcast((64, 512)))
                            nc.vector.tensor_mul(ytiles[hj][:, qoff:qoff + 512],
                                                 py[0:64, :], rcpb[:])
                # --- A2A y ---
                for r in range(NCORES):
                    blk = a2ay_i[r * P:(r + 1) * P, :]
                    nc.sync.dma_start(out=blk[0:64, :], in_=yT0[:, r * OWN:(r + 1) * OWN])
                    nc.sync.dma_start(out=blk[64:128, :], in_=yT1[:, r * OWN:(r + 1) * OWN])
                nc.gpsimd.collective_compute("AllToAll", OP.bypass, replica_groups=rg,
                                             ins=[a2ay_i[:, :]], outs=[a2ay_o[:, :]])

            # ============ proj / norm2 / gating / lb / dispatch ============
            with (
                tc.tile_pool(name="pG", bufs=1) as pg,
                tc.tile_pool(name="pGL", bufs=3) as pgl,
            ):
                with tc.tile_pool(name="psG1", bufs=1, space="PSUM") as psg1:
                    x_sb2 = pg.tile([P, 4, C], F32)
                    nc.sync.dma_start(out=x_sb2[:],
                                      in_=x_own[:, :].rearrange("(t p) c -> p t c", p=P))
                    yo = pg.tile([P, 8, OWN], BF16)
                    nc.sync.dma_start(out=yo[:],
                                      in_=a2ay_o[:, :].rearrange("(k p) t -> p k t", p=P))
                    wp_sb = pg.tile([P, 8, C], BF16)
                    nc.sync.dma_start(out=wp_sb[:],
                                      in_=w_proj_b[:, :].rearrange("(k p) n -> p k n", p=P))
                    bp_sb = pg.tile([1, C], BF16)
                    nc.sync.dma_start(out=bp_sb[:], in_=b_proj[:, :])
                    x2 = pg.tile([P, 4, C], F32)
                    for tt in range(4):
                        for nh in range(2):
                            ps = psg1.tile([P, 512], F32, space="PSUM", tag="proj")
                            for k in range(8):
                                nc.tensor.matmul(ps[:], lhsT=yo[:, k, tt * P:(tt + 1) * P],
                                                 rhs=wp_sb[:, k, nh * 512:(nh + 1) * 512],
                                                 start=(k == 0), stop=False)
                            nc.tensor.matmul(ps[:], lhsT=ones_row[:1, :],
                                             rhs=bp_sb[:1, nh * 512:(nh + 1) * 512],
                                             start=False, stop=True)
                            nc.vector.tensor_add(x2[:, tt, nh * 512:(nh + 1) * 512],
                                                 ps[:], x_sb2[:, tt, nh * 512:(nh + 1) * 512])
                    nc.sync.dma_start(out=x2_out[:, :].rearrange("(t p) c -> p t c", p=P),
                                      in_=x2[:])
                    # rmsnorm2
                    ln2_sb = pg.tile([P, 8], F32)
                    nc.sync.dma_start(out=ln2_sb[:], in_=ln2_wt[:, :])
                    ln2row = pg.tile([P, C], F32)
                    for kb in range(8):
                        nc.sync.dma_start(
                            out=ln2row[:, kb * P:(kb + 1) * P],
                            in_=ln2_wt[:, kb:kb + 1].rearrange("(o p) k -> o (k p)", o=1)
                            .to_broadcast((P, P)))
                    h2 = pg.tile([P, 4, C], BF16)
                    sc2 = pg.tile([P, 4], F32)
                    for tt in range(4):
                        sq2 = pgl.tile([P, C], BF16, tag="sq2")
                        ssq = pgl.tile([P, 1], F32, tag="ssq2")
                        nc.scalar.activation(sq2[:], x2[:, tt, :], AF.Square,
                                             scale=float(1.0 / 32.0), accum_out=ssq[:])
                        lnv = pgl.tile([P, 1], F32, tag="lnv2")
                        nc.scalar.activation(lnv[:], ssq[:], AF.Ln, bias=eps_col[:, :1])
                        nc.scalar.activation(sc2[:, tt:tt + 1], lnv[:], AF.Exp, scale=-0.5)
                        tmp = pgl.tile([P, C], F32, tag="tmpn2")
                        nc.vector.tensor_scalar(out=tmp[:], in0=x2[:, tt, :],
                                                scalar1=sc2[:, tt:tt + 1], scalar2=None,
                                                op0=OP.mult)
                        nc.vector.tensor_mul(h2[:, tt, :], tmp[:], ln2row[:])
                    # transposes: x2 -> x2T (f32), h2 -> h2T (bf16)
                    x2T = pg.tile([P, 8, OWN], F32)
                    h2T = pg.tile([P, 8, OWN], BF16)
                    for tt in range(4):
                        for kb in range(8):
                            tpf = psg1.tile([P, P], F32, space="PSUM", tag="tpf")
                            nc.tensor.transpose(tpf[:], x2[:, tt, kb * P:(kb + 1) * P],
                                                identf[:])
                            nc.scalar.copy(x2T[:, kb, tt * P:(tt + 1) * P], tpf[:])
                            tp2 = psg1.tile([P, P], BF16, space="PSUM", tag="tp2")
                            nc.tensor.transpose(tp2[:], h2[:, tt, kb * P:(kb + 1) * P],
                                                ident[:])
                            nc.scalar.copy(h2T[:, kb, tt * P:(tt + 1) * P], tp2[:])
                    # h2T fp32 (for gating): x2T * sc2_bcast * ln2
                    nc.sync.dma_start(out=sc2_dram[:, :], in_=sc2[:])
                    sc2row = pg.tile([P, 4, P], F32)
                    for tt in range(4):
                        nc.sync.dma_start(
                            out=sc2row[:, tt, :],
                            in_=sc2_dram[:, tt:tt + 1].rearrange("(o p) t -> o (t p)", o=1)
                            .to_broadcast((P, P)))
                    gw_sb = pg.tile([P, 8, E], F32)
                    nc.sync.dma_start(out=gw_sb[:],
                                      in_=gate_wt[:, :].rearrange("(k p) e -> p k e", p=P))
                    h2Tf = pg.tile([P, 8, OWN], F32)
                    for kb in range(8):
                        nc.vector.tensor_mul(
                            h2Tf[:, kb, :].rearrange("p (t q) -> p t q", q=P),
                            x2T[:, kb, :].rearrange("p (t q) -> p t q", q=P), sc2row[:])
                        nc.vector.tensor_scalar(out=h2Tf[:, kb, :], in0=h2Tf[:, kb, :],
                                                scalar1=ln2_sb[:, kb:kb + 1], scalar2=None,
                                                op0=OP.mult)
                    # gating (fp32)
                    wd = pg.tile([P, 4, E], F32)
                    pfull = pg.tile([P, 4, E], F32)
                    for tt in range(4):
                        psg = psg1.tile([P, E], F32, space="PSUM", tag="gate")
                        for k in range(8):
                            nc.tensor.matmul(psg[:], lhsT=h2Tf[:, k, tt * P:(tt + 1) * P],
                                             rhs=gw_sb[:, k, :], start=(k == 0),
                                             stop=(k == 7))
                        pe_ = pgl.tile([P, E], F32, tag="pe")
                        sume = pgl.tile([P, 1], F32, tag="sume")
                        nc.scalar.activation(pe_[:], psg[:], AF.Exp, accum_out=sume[:])
                        rs = pgl.tile([P, 1], F32, tag="rs")
                        nc.vector.reciprocal(rs[:], sume[:])
                        nc.vector.tensor_scalar(out=pfull[:, tt, :], in0=pe_[:],
                                                scalar1=rs[:, :1], scalar2=None, op0=OP.mult)
                        pb = pgl.tile([P, E], F32, tag="pb")
                        nc.vector.tensor_add(pb[:], pfull[:, tt, :], gbias_b[:])
                        m8 = pgl.tile([P, 8], F32, tag="m8")
                        nc.vector.max(out=m8[:], in_=pb[:])
                        msk = pgl.tile([P, E], F32, tag="msk")
                        nc.vector.tensor_scalar(out=msk[:], in0=pb[:], scalar1=m8[:, 1:2],
                                                scalar2=None, op0=OP.is_ge)
                        nc.vector.tensor_mul(wd[:, tt, :], pfull[:, tt, :], msk[:])

                    # collectives: dispatch cols + stats + h2 rows + h2T blocks
                    for r in range(NCORES):
                        nc.sync.dma_start(
                            out=a2aw_i[r * OWN:(r + 1) * OWN, :]
                            .rearrange("(t p) e -> p t e", p=P),
                            in_=wd[:, :, 2 * r:2 * r + 2])
                    nc.gpsimd.collective_compute("AllToAll", OP.bypass, replica_groups=rg,
                                                 ins=[a2aw_i[:, :]], outs=[a2aw_o[:, :]])
                    stv = agst_i[:, :].rearrange("(t p) e -> p t e", p=P)
                    nc.sync.dma_start(out=stv[:, :, 0:E], in_=wd[:])
                    nc.sync.dma_start(out=stv[:, :, E:2 * E], in_=pfull[:])
                    nc.gpsimd.collective_compute("AllGather", OP.bypass, replica_groups=rg,
                                                 ins=[agst_i[:, :]], outs=[agst_o[:, :]])
                    nc.sync.dma_start(out=agh2_i[:, :].rearrange("(t p) c -> p t c", p=P),
                                      in_=h2[:])
                    nc.gpsimd.collective_compute("AllGather", OP.bypass, replica_groups=rg,
                                                 ins=[agh2_i[:, :]], outs=[agh2_o[:, :]])
                    nc.sync.dma_start(out=agh2t_i[:, :].rearrange("(k p) t -> p k t", p=P),
                                      in_=h2T[:])
                    nc.gpsimd.collective_compute("AllGather", OP.bypass, replica_groups=rg,
                                                 ins=[agh2t_i[:, :]], outs=[agh2t_o[:, :]])

                    # lb loss
                    pslb = psg1.tile([1, 2 * E], F32, space="PSUM", tag="lb")
                    for tt in range(32):
                        st = pgl.tile([P, 2 * E], F32, tag="stt")
                        nc.sync.dma_start(out=st[:], in_=agst_o[tt * P:(tt + 1) * P, :])
                        ind = pgl.tile([P, 2 * E], F32, tag="ind")
                        nc.vector.tensor_scalar(out=ind[:, 0:E], in0=st[:, 0:E],
                                                scalar1=0.0, scalar2=None, op0=OP.is_gt)
                        nc.vector.tensor_copy(ind[:, E:2 * E], st[:, E:2 * E])
                        nc.tensor.matmul(pslb[:], lhsT=onesb[:, :1], rhs=ind[:],
                                         start=(tt == 0), stop=(tt == 31))
                    cnt_sp = pg.tile([1, 2 * E], F32)
                    nc.scalar.copy(cnt_sp[:], pslb[:])
                    prod = pg.tile([1, E], F32)
                    nc.vector.tensor_mul(prod[:], cnt_sp[:1, 0:E], cnt_sp[:1, E:2 * E])
                    lbv = pg.tile([1, 1], F32)
                    nc.vector.reduce_sum(out=lbv[:], in_=prod[:], axis=AX.X)
                    lbs = pg.tile([1, 1], F32)
                    nc.scalar.activation(lbs[:], lbv[:], AF.Copy,
                                         scale=float(ALPHA * E / ((TOPK * NTOK + 1e-6) * NTOK)))
                    nc.sync.dma_start(out=lb_out[:, :], in_=lbs[:])

                # dispatch lists (per local expert)
                with tc.tile_pool(name="psJ", bufs=1, space="PSUM") as psj:
                    idw = pg.tile([P, 32, P], F32)
                    nc.vector.memset(idw[:], 0.0)
                    nc.vector.tensor_copy(idw[:, :, 0], iota_tok[:])
                    for e in range(EPC):
                        wcol = pgl.tile([P, 32], F32, tag="wcol")
                        nc.sync.dma_start(
                            out=wcol[:],
                            in_=a2aw_o[:, :].rearrange("(p f) e -> p f e", f=32)[:, :, e])
                        msk = pgl.tile([P, 32], F32, tag="dmask")
                        nc.vector.tensor_scalar(out=msk[:], in0=wcol[:], scalar1=0.0,
                                                scalar2=None, op0=OP.is_gt)
                        zer = pgl.tile([P, 32], F32, tag="dzer")
                        nc.vector.memset(zer[:], 0.0)
                        incl = pgl.tile([P, 32], F32, tag="dincl")
                        nc.vector.tensor_tensor_scan(incl[:], zer[:], msk[:], 0.0,
                                                     op0=OP.add, op1=OP.add)
                        poslocal = pgl.tile([P, 32], F32, tag="dpos")
                        nc.vector.tensor_sub(poslocal[:], incl[:], msk[:])
                        rc = pgl.tile([P, 1], F32, tag="drc")
                        nc.vector.reduce_sum(out=rc[:], in_=msk[:], axis=AX.X)
                        prep = psj.tile([P, 1], F32, space="PSUM", tag="dpre")
                        nc.tensor.matmul(prep[:], lhsT=ustrict[:], rhs=rc[:],
                                         start=True, stop=True)
                        pref = pgl.tile([P, 1], F32, tag="dpref")
                        nc.vector.tensor_copy(pref[:], prep[:])
                        slot = pgl.tile([P, 32], F32, tag="dslot")
                        nc.vector.tensor_scalar(out=slot[:], in0=poslocal[:],
                                                scalar1=pref[:, :1], scalar2=None,
                                                op0=OP.add)
                        nc.vector.tensor_mul(slot[:], slot[:], msk[:])
                        nc.vector.tensor_add(slot[:], slot[:], msk[:])
                        nc.vector.tensor_scalar_add(slot[:], slot[:], -1.0)
                        nc.vector.tensor_copy(idw[:, :, 1], wcol[:])
                        lps = []
                        for j in range(CAPT):
                            lp_j = psj.tile([P, P], F32, space="PSUM", tag=f"dl{j}")
                            lps.append(lp_j)
                        for f in range(32):
                            a_f = pgl.tile([P, CAP], F32, tag="af")
                            nc.vector.tensor_tensor(
                                out=a_f[:], in0=slot[:, f:f + 1].to_broadcast([P, CAP]),
                                in1=iota_cap[:], op=OP.is_equal)
                            for j in range(CAPT):
                                nc.tensor.matmul(lps[j][:], lhsT=idw[:, f, :],
                                                 rhs=a_f[:, j * P:(j + 1) * P],
                                                 start=(f == 0), stop=(f == 31))
                        lst = pgl.tile([2, CAP], F32, tag="lst")
                        for j in range(CAPT):
                            nc.scalar.copy(lst[:, j * P:(j + 1) * P], lps[j][:2, :])
                        nc.sync.dma_start(out=ids_dram[None, e * CAP:(e + 1) * CAP],
                                          in_=lst[0:1, :])
                        nc.sync.dma_start(out=wsl_dram[None, e * CAP:(e + 1) * CAP],
                                          in_=lst[1:2, :])

            # ============ shared expert (writes out_partial first) ============
            with (
                tc.tile_pool(name="pL", bufs=1) as sp,
                tc.tile_pool(name="pLl", bufs=2) as spl,
                tc.tile_pool(name="psL", bufs=2, space="PSUM") as psl,
            ):
                swf = sp.tile([P, 8, HIDS], BF16)
                nc.sync.dma_start(out=swf[:],
                                  in_=sh_wf_b[:, :].rearrange("(k p) n -> p k n", p=P))
                swp = sp.tile([P, 2, C], BF16)
                nc.sync.dma_start(out=swp[:],
                                  in_=sh_wp_b[:, :].rearrange("(k p) n -> p k n", p=P))
                sbf = sp.tile([P, 2], F32)
                nc.sync.dma_start(out=sbf[:], in_=sh_bf_t[:, :])
                sbp = sp.tile([1, C], BF16)
                nc.sync.dma_start(out=sbp[:], in_=sh_bp8[:, :])
                for r in range(NCORES):
                    hblk = spl.tile([P, 8, OWN], BF16, tag="hblk")
                    nc.sync.dma_start(
                        out=hblk[:],
                        in_=agh2t_o[r * C:(r + 1) * C, :].rearrange("(k p) t -> p k t", p=P))
                    s_sh = spl.tile([P, 2, OWN], BF16, tag="ssh")
                    for ht in range(2):
                        ps1 = psl.tile([P, 512], F32, space="PSUM", tag="sl1")
                        for k in range(8):
                            nc.tensor.matmul(ps1[:], lhsT=swf[:, k, ht * P:(ht + 1) * P],
                                             rhs=hblk[:, k, :], start=(k == 0),
                                             stop=(k == 7))
                        nc.scalar.activation(s_sh[:, ht, :], ps1[:], AF.Silu,
                                             bias=sbf[:, ht:ht + 1])
                    for tt in range(4):
                        for nh in range(2):
                            ps2 = psl.tile([P, 512], F32, space="PSUM", tag="sl2")
                            for k in range(2):
                                nc.tensor.matmul(ps2[:],
                                                 lhsT=s_sh[:, k, tt * P:(tt + 1) * P],
                                                 rhs=swp[:, k, nh * 512:(nh + 1) * 512],
                                                 start=(k == 0), stop=False)
                            nc.tensor.matmul(ps2[:], lhsT=ones_row[:1, :],
                                             rhs=sbp[:1, nh * 512:(nh + 1) * 512],
                                             start=False, stop=True)
                            osh = spl.tile([P, 512], F32, tag="osh")
                            nc.scalar.copy(osh[:], ps2[:])
                            nc.sync.dma_start(
                                out=out_partial[r * OWN + tt * P: r * OWN + (tt + 1) * P,
                                                nh * 512:(nh + 1) * 512],
                                in_=osh[:])

            # ============ routed experts (scatter-add after shared) ============
            with (
                tc.tile_pool(name="pK", bufs=1) as ep,
                tc.tile_pool(name="pKl", bufs=2) as epl,
                tc.tile_pool(name="wfp", bufs=2) as wfp,
                tc.tile_pool(name="wpp", bufs=1) as wpp,
                tc.tile_pool(name="psK", bufs=1, space="PSUM") as pse,
            ):
                bf_sb = ep.tile([P, EPC * 16], F32)
                nc.sync.dma_start(out=bf_sb[:], in_=exp_bf_t[:, :])
                for e in range(EPC):
                    wf = wfp.tile([P, 8, 2 * C], BF16, tag="wf")
                    nc.sync.dma_start(out=wf[:],
                                      in_=exp_wf_b[e, :, :].rearrange("(k p) n -> p k n", p=P))
                    wp2 = wpp.tile([P, 16, C], BF16, tag="wp")
                    nc.sync.dma_start(out=wp2[:],
                                      in_=exp_wp_b[e, :, :].rearrange("(k p) n -> p k n", p=P))
                    bp_e = epl.tile([1, C], BF16, tag="bpe")
                    nc.sync.dma_start(out=bp_e[:], in_=exp_bp[e:e + 1, :])
                    idxt, wst = [], []
                    for t in range(CAPT):
                        idxf = epl.tile([P, 1], F32, tag=f"idxf{t}")
                        nc.sync.dma_start(
                            out=idxf[:],
                            in_=ids_dram[e * CAP + t * P: e * CAP + (t + 1) * P, None])
                        ii = epl.tile([P, 1], I32, tag=f"idxi{t}")
                        nc.vector.tensor_copy(ii[:], idxf[:])
                        idxt.append(ii)
                        ws = epl.tile([P, 1], F32, tag=f"wst{t}")
                        nc.sync.dma_start(
                            out=ws[:],
                            in_=wsl_dram[e * CAP + t * P: e * CAP + (t + 1) * P, None])
                        wst.append(ws)
                    x_eT = ep.tile([P, 8, CAP], BF16, tag="xeT")
                    for t in range(CAPT):
                        g = epl.tile([P, C], BF16, tag="gat")
                        nc.gpsimd.indirect_dma_start(
                            out=g[:], out_offset=None, in_=agh2_o[:, :],
                            in_offset=bass.IndirectOffsetOnAxis(ap=idxt[t][:, :1], axis=0))
                        for kb in range(8):
                            tp = pse.tile([P, P], BF16, space="PSUM", tag="etp")
                            nc.tensor.transpose(tp[:], g[:, kb * P:(kb + 1) * P], ident[:])
                            nc.scalar.copy(x_eT[:, kb, t * P:(t + 1) * P], tp[:])
                    s_e = ep.tile([P, 16, CAP], BF16, tag="se")
                    for hh in range(16):
                        for j, (n0, n1) in enumerate(((0, 512), (512, 640))):
                            ps1 = pse.tile([P, n1 - n0], F32, space="PSUM", tag=f"el1_{j}")
                            for k in range(8):
                                nc.tensor.matmul(ps1[:], lhsT=wf[:, k, hh * P:(hh + 1) * P],
                                                 rhs=x_eT[:, k, n0:n1],
                                                 start=(k == 0), stop=(k == 7))
                            nc.scalar.activation(s_e[:, hh, n0:n1], ps1[:], AF.Silu,
                                                 bias=bf_sb[:, e * 16 + hh:e * 16 + hh + 1])
                    for t in range(CAPT):
                        oe = epl.tile([P, C], F32, tag="oe")
                        for nh in range(2):
                            ps2 = pse.tile([P, 512], F32, space="PSUM", tag="el2")
                            for k in range(16):
                                nc.tensor.matmul(ps2[:], lhsT=s_e[:, k, t * P:(t + 1) * P],
                                                 rhs=wp2[:, k, nh * 512:(nh + 1) * 512],
                                                 start=(k == 0), stop=False)
                            nc.tensor.matmul(ps2[:], lhsT=ones_row[:1, :],
                                             rhs=bp_e[:1, nh * 512:(nh + 1) * 512],
                                             start=False, stop=True)
                            nc.vector.tensor_scalar(out=oe[:, nh * 512:(nh + 1) * 512],
                                                    in0=ps2[:], scalar1=wst[t][:, :1],
                                                    scalar2=None, op0=OP.mult)
                        nc.gpsimd.indirect_dma_start(
                            out=out_partial[:, :],
                            out_offset=bass.IndirectOffsetOnAxis(ap=idxt[t][:, :1], axis=0),
                            in_=oe[:], in_offset=None,
                            compute_op=OP.add)

    nc.compile()
    return nc


_NC_CACHE = None


def _get_nc():
    global _NC_CACHE
    if _NC_CACHE is None:
        _NC_CACHE = build_nc()
    return _NC_CACHE


def kernel(x, ln1_w, ln2_w, w_attn, b_attn, w_proj, b_proj, gate_w, gate_bias,
           exp_wf, exp_bf, exp_wp, exp_bp, sh_wf, sh_bf, sh_wp, sh_bp):
    bf = ml_dtypes.bfloat16
    x = np.asarray(x, np.float32)
    xf = np.ascontiguousarray(x.reshape(NTOK, C))

    theta = 10000.0
    freqs = 1.0 / theta ** (np.arange(0, DH // 2, dtype=np.float32) * 2.0 / DH)
    ang = np.arange(T, dtype=np.float32)[:, None] * freqs[None, :]
    cos_t = np.cos(ang).astype(np.float32)
    sin_t = np.sin(ang).astype(np.float32)

    w_attn_b = np.ascontiguousarray(np.asarray(w_attn, np.float32)).astype(bf)
    w_proj_b = np.ascontiguousarray(np.asarray(w_proj, np.float32)).astype(bf)
    gate_wt = np.ascontiguousarray(np.asarray(gate_w, np.float32).T)
    exp_wf_b = np.asarray(exp_wf, np.float32).astype(bf)
    exp_wp_b = np.asarray(exp_wp, np.float32).astype(bf)
    sh_wf_b = np.asarray(sh_wf, np.float32).astype(bf)
    sh_wp_b = np.asarray(sh_wp, np.float32).astype(bf)
    exp_bf_f = np.asarray(exp_bf, np.float32)
    sh_bf_f = np.asarray(sh_bf, np.float32)

    in_maps = []
    for c in range(NCORES):
        tok0 = c * OWN
        tglob = (np.arange(OWN) + tok0) % T
        im = {
            "x_own": np.ascontiguousarray(xf[tok0:tok0 + OWN]),
            "ln1_w": np.asarray(ln1_w, np.float32).reshape(1, C),
            "ln2_wt": np.ascontiguousarray(
                np.asarray(ln2_w, np.float32).reshape(8, P).T),
            "w_attn_b": w_attn_b,
            "b_attn": np.asarray(b_attn, np.float32).reshape(1, 3 * C).astype(bf),
            "w_proj_b": w_proj_b,
            "b_proj": np.asarray(b_proj, np.float32).reshape(1, C).astype(bf),
            "gate_wt": gate_wt,
            "gate_bias": np.asarray(gate_bias, np.float32).reshape(1, E),
            "cos_own": np.ascontiguousarray(cos_t[tglob]),
            "sin_own": np.ascontiguousarray(sin_t[tglob]),
            "exp_wf_b": np.ascontiguousarray(exp_wf_b[2 * c:2 * c + 2]),
            "exp_wp_b": np.ascontiguousarray(exp_wp_b[2 * c:2 * c + 2]),
            "exp_bf_t": np.ascontiguousarray(
                exp_bf_f[2 * c:2 * c + 2].reshape(2, 16, P)
                .transpose(2, 0, 1).reshape(P, 32)),
            "exp_bp": np.ascontiguousarray(
                np.asarray(exp_bp, np.float32)[2 * c:2 * c + 2]).astype(bf),
            "sh_wf_b": np.ascontiguousarray(sh_wf_b[:, c * HIDS:(c + 1) * HIDS]),
            "sh_wp_b": np.ascontiguousarray(sh_wp_b[c * HIDS:(c + 1) * HIDS, :]),
            "sh_bf_t": np.ascontiguousarray(
                sh_bf_f[c * HIDS:(c + 1) * HIDS].reshape(2, P).T),
            "sh_bp8": (np.asarray(sh_bp, np.float32) / NCORES).reshape(1, C).astype(bf),
        }
        in_maps.append(im)

    nc = _get_nc()
    res = run_bass_kernel_spmd(nc, in_maps, core_ids=list(range(NCORES)))

    out = np.zeros((NTOK, C), np.float32)
    for c in range(NCORES):
        out += res.results[c]["out_partial"]
        out[c * OWN:(c + 1) * OWN] += res.results[c]["x2_out"]
    lb = np.float32(res.results[0]["lb_out"][0, 0])
    return out.reshape(B, T, C), lb
